# revision 1
# baseline (speedup 1.0000x reference)
"""Trainium2 Bass kernel for DatasetIndexedTopK (streaming top-k retrieval).

Problem: scores = Q @ C^T with Q [512, 128], C [1M, 128]; return per-query
top-100 (scores, ids), matching jax.lax.top_k semantics (ties -> lower id).

Design (8-way shard over candidates, 131072 per core):
  Device (per core): stream candT in 8192-wide tiles; f32r matmuls (full PE
  rate, max |score err| ~9e-3 measured) fill [128, 2048] PSUM tiles; the DVE
  reduces each PSUM tile directly (tensor_reduce max over innermost 32) into
  a bf16 cell-max summary S [128, 4096] per 128-query chunk.  Only the
  per-32-candidate maxima leave the device: out_cm [512, 4096] bf16.

  Host: concat the 8 cores' summaries -> [512, 32768] cell maxima.  The
  100th-largest cell-max v100 >= s_100 - 0.26 (bf16 rounding 0.125 + f32r
  error 0.01, both ways), so every cell containing a true top-100 candidate
  has summary >= v100 - 0.52.  Select cells >= v100 - 0.7 (provably a
  superset), gather their 32 candidates each, rescore exactly in fp32, and
  take the exact top-k with the reference's tie order (score desc, id asc).
"""

import numpy as np

P = 128                 # SBUF partitions / queries per chunk
D = 128                 # embedding dim (contraction)
Q = 512                 # queries
NCORES = 8
NCAND_TOTAL = 256 * 4096
NCAND = NCAND_TOTAL // NCORES    # 131072 candidates per core
CTILE = 8192            # candidate columns per DMA tile
PBLK = 2048             # columns per PSUM tile (4 banks)
CELL = 32               # candidates per summary cell
NCELL = NCAND // CELL   # 4096 cells per core
NCHUNK = Q // P         # 4 query chunks
MARGIN = 1.0            # cell-selection slack (> 2*(mm err 0.19 + bf16 ulp 0.13))

_CACHE = {}


def _build_bass(repeat=1):
    import concourse.bacc as bacc
    import concourse.mybir as mybir
    from concourse.tile import TileContext
    from contextlib import ExitStack

    f32 = mybir.dt.float32
    bf16 = mybir.dt.bfloat16
    ncell_span = PBLK // CELL          # 64
    nspan = CTILE // PBLK              # 4

    nc = bacc.Bacc()
    qT = nc.declare_dram_parameter("qT", [D, Q], bf16, isOutput=False)
    candT = nc.declare_dram_parameter("candT", [D, NCAND], bf16, isOutput=False)
    out_cm = nc.declare_dram_parameter("out_cm", [Q, NCELL], bf16, isOutput=True)

    with ExitStack() as ctx:
        tc = ctx.enter_context(TileContext(nc))
        qpool = ctx.enter_context(tc.tile_pool(name="q", bufs=1))
        cpool = ctx.enter_context(tc.tile_pool(name="cand", bufs=4))
        pspool = ctx.enter_context(tc.tile_pool(name="ps", bufs=2, space="PSUM"))
        acc = ctx.enter_context(tc.tile_pool(name="acc", bufs=1))

        qsb = qpool.tile([D, Q], bf16, tag="qsb")
        nc.sync.dma_start(qsb[:], qT[:])

        S_all = acc.tile([P, NCHUNK * NCELL], bf16, tag="S")

        for t in range(repeat * (NCAND // CTILE)):
            t = t % (NCAND // CTILE)
            ct = cpool.tile([D, CTILE], bf16, tag="cand")
            nc.sync.dma_start(ct[:], candT[:, t * CTILE:(t + 1) * CTILE])
            for qc in range(NCHUNK):
                for sp in range(nspan):
                    ps = pspool.tile([P, ncell_span, CELL], f32, tag="ps")
                    for j in range(PBLK // 512):
                        col = sp * PBLK + j * 512
                        npc = 512 // CELL
                        nc.tensor.matmul(
                            ps[:, j * npc:(j + 1) * npc, :],
                            lhsT=qsb[:, qc * P:(qc + 1) * P],
                            rhs=ct[:, col: col + 512],
                            start=True,
                            stop=True,
                        )
                    so = qc * NCELL + t * (CTILE // CELL) + sp * ncell_span
                    nc.vector.tensor_reduce(
                        out=S_all[:, so:so + ncell_span], in_=ps[:],
                        axis=mybir.AxisListType.X, op=mybir.AluOpType.max,
                    )

        for qc in range(NCHUNK):
            nc.sync.dma_start(
                out_cm[qc * P:(qc + 1) * P, :],
                S_all[:, qc * NCELL:(qc + 1) * NCELL],
            )
    nc.compile()
    return nc


def _get_bass():
    if "nc" not in _CACHE:
        _CACHE["nc"] = _build_bass()
    return _CACHE["nc"]


def kernel(query_embeddings, candidate_embeddings, candidate_indices, k):
    from concourse.bass_utils import run_bass_kernel_spmd

    q = np.ascontiguousarray(np.asarray(query_embeddings, dtype=np.float32))
    c = np.asarray(candidate_embeddings, dtype=np.float32).reshape(NCAND_TOTAL, D)
    ids_flat = np.asarray(candidate_indices).reshape(-1)
    k = int(k)
    assert k <= 1024

    import ml_dtypes
    bf16 = ml_dtypes.bfloat16
    qT = np.ascontiguousarray(q.T).astype(bf16)          # [128, 512]
    cT = np.ascontiguousarray(c.T.astype(bf16))          # [128, 1048576]
    in_maps = []
    for core in range(NCORES):
        in_maps.append({
            "qT": qT,
            "candT": cT[:, core * NCAND:(core + 1) * NCAND],
        })

    nc = _get_bass()
    res = run_bass_kernel_spmd(nc, in_maps, core_ids=list(range(NCORES))).results

    # ---- host: exact top-k from cell-max summaries ----
    cm = np.concatenate(
        [res[core]["out_cm"].astype(np.float32) for core in range(NCORES)],
        axis=1,
    )                                                    # [512, 32768]
    vk = np.partition(cm, -k, axis=1)[:, -k]             # kth-largest cell max
    tau = vk - MARGIN
    counts = (cm >= tau[:, None]).sum(axis=1)
    K = int(counts.max())
    sel_cells = np.argpartition(-cm, K - 1, axis=1)[:, :K]   # [512, K]

    # global candidate positions of each selected cell's 32 members
    core_of = sel_cells >> 12                            # // 4096
    local = sel_cells & 0xFFF
    base = core_of * NCAND + local * CELL                # [512, K]
    pos = (base[:, :, None] + np.arange(CELL)[None, None, :]).reshape(Q, K * CELL)

    out_scores = np.empty((Q, k), dtype=np.float32)
    out_pos = np.empty((Q, k), dtype=np.int64)
    QB = 64                                              # query batch (memory cap)
    for q0 in range(0, Q, QB):
        q1 = min(q0 + QB, Q)
        sel = c[pos[q0:q1]]                              # [qb, K*32, 128]
        sc = np.einsum("qnd,qd->qn", sel, q[q0:q1], optimize=True)
        for qi in range(q0, q1):
            row = sc[qi - q0]
            p = pos[qi]
            # exact order among a slightly larger head to honor tie-break
            head = np.argpartition(-row, min(k + 32, row.size - 1))[:k + 32]
            order = head[np.lexsort((p[head], -row[head]))][:k]
            out_scores[qi] = row[order]
            out_pos[qi] = p[order]

    out_ids = ids_flat[out_pos].astype(ids_flat.dtype)
    return out_scores, out_ids



# revision 3
# speedup vs baseline: 1.4439x; 1.4439x over previous
"""Trainium2 Bass kernel for DatasetIndexedTopK (streaming top-k retrieval).

Problem: scores = Q @ C^T with Q [512, 128], C [1M, 128]; return per-query
top-100 (scores, ids), matching jax.lax.top_k semantics (ties -> lower id).

Design (8-way shard over candidates, 131072 per core):
  The score volume per core is 4 query-chunks x 131072 candidates = 524288
  elements per SBUF partition; the bottleneck is draining it from PSUM
  (DVE tensor_reduce is 1x-rate, ~0.96 GHz).  v2 splits the drain between
  two engines working in parallel:

    span 0 of each 8192-wide tile (2048 cols):  DVE tensor_reduce max over
        innermost 32 directly from PSUM -> bf16 cell-max, cells = 32
        consecutive candidates.                         (~1.04 ns/el on DVE)
    spans 1..3: ACT (scalar engine) copies PSUM f32 -> SBUF bf16
        (~0.83 ns/el on ACT), then DVE folds with tensor_max (bf16 SBUF
        hits the 2x perf mode, ~0.52 ns/el) into a running per-column max
        across the 16 tiles -> cells = 16 candidates strided by 8192.

  Only cell maxima leave the device: out_cm [512, NCELL] bf16 per core
  (region A: 1024 32-member cells per chunk; region B: 6144 16-member
  cells per chunk).

  Host: concat the 8 cores' summaries; the kth-largest cell-max vk bounds
  the true s_k within the (matmul + bf16-rounding) margin, so selecting
  cells >= vk - MARGIN provably covers the true top-k.  Gather members of
  selected cells, rescore exactly in fp32, take exact top-k with the
  reference tie order (score desc, id asc).
"""

import numpy as np

P = 128                 # SBUF partitions / queries per chunk
D = 128                 # embedding dim (contraction)
Q = 512                 # queries
NCORES = 8
NCAND_TOTAL = 256 * 4096
NCAND = NCAND_TOTAL // NCORES    # 131072 candidates per core
CTILE = 8192            # candidate columns per DMA tile
NTILE = NCAND // CTILE  # 16
SPAN = 2048             # columns per PSUM span (4 banks)
NSPAN = CTILE // SPAN   # 4 spans per tile
S_A = 1                 # spans per tile drained by DVE tensor_reduce
N_B = NSPAN - S_A       # spans per tile drained via ACT copy + DVE fold
CELL_A = 32             # members per A-cell (consecutive)
CELL_B = NTILE          # members per B-cell (strided by CTILE)
NA = NTILE * S_A * (SPAN // CELL_A)   # A-cells per chunk  (1024)
NB = N_B * SPAN                        # B-cells per chunk  (6144)
NCELL = NA + NB                        # summary width per chunk (7168)
NCHUNK = Q // P         # 4 query chunks
MARGIN = 1.0            # cell-selection slack (>> mm err + 2x bf16 ulp)

_CACHE = {}


def _build_bass(repeat=1):
    import concourse.bacc as bacc
    import concourse.mybir as mybir
    from concourse.tile import TileContext
    from contextlib import ExitStack

    f32 = mybir.dt.float32
    bf16 = mybir.dt.bfloat16
    ncell_span = SPAN // CELL_A        # 64

    nc = bacc.Bacc()
    qT = nc.declare_dram_parameter("qT", [D, Q], bf16, isOutput=False)
    candT = nc.declare_dram_parameter("candT", [D, NCAND], bf16, isOutput=False)
    out_cm = nc.declare_dram_parameter("out_cm", [Q, NCELL], bf16, isOutput=True)

    with ExitStack() as ctx:
        tc = ctx.enter_context(TileContext(nc))
        qpool = ctx.enter_context(tc.tile_pool(name="q", bufs=1))
        cpool = ctx.enter_context(tc.tile_pool(name="cand", bufs=3))
        pspool = ctx.enter_context(tc.tile_pool(name="ps", bufs=2, space="PSUM"))
        apool = ctx.enter_context(tc.tile_pool(name="accA", bufs=1))
        bpool = ctx.enter_context(tc.tile_pool(name="accB", bufs=1))
        stgpool = ctx.enter_context(tc.tile_pool(name="stg", bufs=4))

        qsb = qpool.tile([D, Q], bf16, tag="qsb")
        nc.sync.dma_start(qsb[:], qT[:])

        # A-region cell maxima: [128, NCHUNK * NA] bf16
        SA = apool.tile([P, NCHUNK * NA], bf16, tag="SA")
        # B-region running maxima: one [128, 64, 32] tile per (chunk, slot)
        accB = [
            [bpool.tile([P, ncell_span, CELL_A], bf16,
                        tag=f"accB_{qc}_{sl}", name=f"accB_{qc}_{sl}")
             for sl in range(N_B)]
            for qc in range(NCHUNK)
        ]

        for t in range(repeat * NTILE):
            t = t % NTILE
            ct = cpool.tile([D, CTILE], bf16, tag="cand")
            nc.sync.dma_start(ct[:], candT[:, t * CTILE:(t + 1) * CTILE])
            for qc in range(NCHUNK):
                for sp in range(NSPAN):
                    ps = pspool.tile([P, ncell_span, CELL_A], f32, tag="ps")
                    for j in range(SPAN // 512):
                        col = sp * SPAN + j * 512
                        npc = 512 // CELL_A
                        nc.tensor.matmul(
                            ps[:, j * npc:(j + 1) * npc, :],
                            lhsT=qsb[:, qc * P:(qc + 1) * P],
                            rhs=ct[:, col: col + 512],
                            start=True,
                            stop=True,
                        )
                    if sp < S_A:
                        so = qc * NA + t * (S_A * ncell_span) + sp * ncell_span
                        nc.vector.tensor_reduce(
                            out=SA[:, so:so + ncell_span], in_=ps[:],
                            axis=mybir.AxisListType.X, op=mybir.AluOpType.max,
                        )
                    else:
                        sl = sp - S_A
                        acc = accB[qc][sl]
                        if t == 0:
                            nc.scalar.activation(
                                acc[:], ps[:],
                                mybir.ActivationFunctionType.Copy,
                            )
                        else:
                            stg = stgpool.tile(
                                [P, ncell_span, CELL_A], bf16, tag="stg")
                            nc.scalar.activation(
                                stg[:], ps[:],
                                mybir.ActivationFunctionType.Copy,
                            )
                            nc.vector.tensor_max(acc[:], acc[:], stg[:])

        for qc in range(NCHUNK):
            nc.sync.dma_start(
                out_cm[qc * P:(qc + 1) * P, :NA],
                SA[:, qc * NA:(qc + 1) * NA],
            )
            for sl in range(N_B):
                c0 = NA + sl * SPAN
                nc.sync.dma_start(
                    out_cm[qc * P:(qc + 1) * P, c0:c0 + SPAN],
                    accB[qc][sl][:],
                )
    nc.compile()
    return nc


def _get_bass():
    if "nc" not in _CACHE:
        _CACHE["nc"] = _build_bass()
    return _CACHE["nc"]


def _cell_member_tables():
    """POS_A [NA, 32] and POS_B_pad [NB, 32] of core-local candidate columns
    (POS_B is padded from 16 to 32 with a huge sentinel)."""
    i = np.arange(NA)
    t = i // (S_A * (SPAN // CELL_A))
    r = i % (S_A * (SPAN // CELL_A))
    sp = r // (SPAN // CELL_A)
    c32 = r % (SPAN // CELL_A)
    baseA = t * CTILE + sp * SPAN + c32 * CELL_A
    POS_A = baseA[:, None] + np.arange(CELL_A)[None, :]

    j = np.arange(NB)
    sl = j // SPAN
    off = j % SPAN
    span = S_A + sl
    POS_B = (np.arange(NTILE)[None, :] * CTILE + (span * SPAN + off)[:, None])
    pad = np.full((NB, CELL_A - CELL_B), 1 << 40, dtype=np.int64)
    POS_B_pad = np.concatenate([POS_B.astype(np.int64), pad], axis=1)
    return POS_A.astype(np.int64), POS_B_pad


def kernel(query_embeddings, candidate_embeddings, candidate_indices, k):
    from concourse.bass_utils import run_bass_kernel_spmd

    q = np.ascontiguousarray(np.asarray(query_embeddings, dtype=np.float32))
    c = np.asarray(candidate_embeddings, dtype=np.float32).reshape(NCAND_TOTAL, D)
    ids_flat = np.asarray(candidate_indices).reshape(-1)
    k = int(k)
    assert k <= 1024

    import ml_dtypes
    bf16 = ml_dtypes.bfloat16
    qT = np.ascontiguousarray(q.T).astype(bf16)          # [128, 512]
    cT = np.ascontiguousarray(c.T.astype(bf16))          # [128, 1048576]
    in_maps = []
    for core in range(NCORES):
        in_maps.append({
            "qT": qT,
            "candT": cT[:, core * NCAND:(core + 1) * NCAND],
        })

    nc = _get_bass()
    res = run_bass_kernel_spmd(nc, in_maps, core_ids=list(range(NCORES))).results

    # ---- host: exact top-k from cell-max summaries ----
    cm = np.concatenate(
        [res[core]["out_cm"].astype(np.float32) for core in range(NCORES)],
        axis=1,
    )                                                    # [512, 8*NCELL]
    vk = np.partition(cm, -k, axis=1)[:, -k]             # kth-largest cell max
    tau = vk - MARGIN
    counts = (cm >= tau[:, None]).sum(axis=1)
    K = int(counts.max())
    sel_cells = np.argpartition(-cm, K - 1, axis=1)[:, :K]   # [512, K]

    POS_A, POS_B_pad = _cell_member_tables()
    SENT = NCAND_TOTAL                                   # dummy candidate id
    core_of = sel_cells // NCELL
    local = sel_cells - core_of * NCELL
    mA = local < NA
    pos = np.empty((Q, K, CELL_A), dtype=np.int64)
    pos[mA] = core_of[mA][:, None] * NCAND + POS_A[local[mA]]
    mB = ~mA
    pos[mB] = core_of[mB][:, None] * NCAND + POS_B_pad[local[mB] - NA]
    np.minimum(pos, SENT, out=pos)
    pos = pos.reshape(Q, K * CELL_A)

    c_ext = np.vstack([c, np.zeros((1, D), dtype=np.float32)])
    out_scores = np.empty((Q, k), dtype=np.float32)
    out_pos = np.empty((Q, k), dtype=np.int64)
    QB = 64                                              # query batch (memory cap)
    for q0 in range(0, Q, QB):
        q1 = min(q0 + QB, Q)
        sel = c_ext[pos[q0:q1]]                          # [qb, K*32, 128]
        sc = np.einsum("qnd,qd->qn", sel, q[q0:q1], optimize=True)
        for qi in range(q0, q1):
            row = sc[qi - q0]
            p = pos[qi]
            # exact order among a slightly larger head to honor tie-break
            head = np.argpartition(-row, min(k + 32, row.size - 1))[:k + 32]
            order = head[np.lexsort((p[head], -row[head]))][:k]
            out_scores[qi] = row[order]
            out_pos[qi] = p[order]

    ids_ext = np.concatenate([ids_flat, np.zeros(1, dtype=ids_flat.dtype)])
    out_ids = ids_ext[out_pos].astype(ids_flat.dtype)
    return out_scores, out_ids


# revision 5
# speedup vs baseline: 1.4510x; 1.0049x over previous
"""Trainium2 Bass kernel for DatasetIndexedTopK (streaming top-k retrieval).

Problem: scores = Q @ C^T with Q [512, 128], C [1M, 128]; return per-query
top-100 (scores, ids), matching jax.lax.top_k semantics (ties -> lower id).

Design (8-way shard over candidates, 131072 per core):
  The score volume per core is 4 query-chunks x 131072 candidates = 524288
  elements per SBUF partition; the bottleneck is draining it from PSUM
  (DVE tensor_reduce is 1x-rate, ~0.96 GHz).  v2 splits the drain between
  two engines working in parallel:

    span 0 of each 8192-wide tile (2048 cols):  DVE tensor_reduce max over
        innermost 32 directly from PSUM -> bf16 cell-max, cells = 32
        consecutive candidates.                         (~1.04 ns/el on DVE)
    spans 1..3: ACT (scalar engine) copies PSUM f32 -> SBUF bf16
        (~0.83 ns/el on ACT), then DVE folds with tensor_max (bf16 SBUF
        hits the 2x perf mode, ~0.52 ns/el) into a running per-column max
        across the 16 tiles -> cells = 16 candidates strided by 8192.

  Only cell maxima leave the device: out_cm [512, NCELL] bf16 per core
  (region A: 1024 32-member cells per chunk; region B: 6144 16-member
  cells per chunk).

  Host: concat the 8 cores' summaries; the kth-largest cell-max vk bounds
  the true s_k within the (matmul + bf16-rounding) margin, so selecting
  cells >= vk - MARGIN provably covers the true top-k.  Gather members of
  selected cells, rescore exactly in fp32, take exact top-k with the
  reference tie order (score desc, id asc).
"""

import numpy as np

P = 128                 # SBUF partitions / queries per chunk
D = 128                 # embedding dim (contraction)
Q = 512                 # queries
NCORES = 8
NCAND_TOTAL = 256 * 4096
NCAND = NCAND_TOTAL // NCORES    # 131072 candidates per core
CTILE = 8192            # candidate columns per DMA tile
NTILE = NCAND // CTILE  # 16
SPAN = 2048             # columns per PSUM span (4 banks)
NSPAN = CTILE // SPAN   # 4 spans per tile
S_A = 1                 # spans per tile drained by DVE tensor_reduce
N_B = NSPAN - S_A       # spans per tile drained via ACT copy + DVE fold
CELL_A = 32             # members per A-cell (consecutive)
CELL_B = NTILE          # members per B-cell (strided by CTILE)
NA = NTILE * S_A * (SPAN // CELL_A)   # A-cells per chunk  (1024)
NB = N_B * SPAN                        # B-cells per chunk  (6144)
NCELL = NA + NB                        # summary width per chunk (7168)
NCHUNK = Q // P         # 4 query chunks
MARGIN = 1.0            # cell-selection slack (>> mm err + 2x bf16 ulp)

_CACHE = {}


def _build_bass(repeat=1):
    import concourse.bacc as bacc
    import concourse.mybir as mybir
    from concourse.tile import TileContext
    from contextlib import ExitStack

    f32 = mybir.dt.float32
    bf16 = mybir.dt.bfloat16
    ncell_span = SPAN // CELL_A        # 64

    nc = bacc.Bacc()
    qT = nc.declare_dram_parameter("qT", [D, Q], bf16, isOutput=False)
    candT = nc.declare_dram_parameter("candT", [D, NCAND], bf16, isOutput=False)
    out_cm = nc.declare_dram_parameter("out_cm", [Q, NCELL], bf16, isOutput=True)

    with ExitStack() as ctx:
        tc = ctx.enter_context(TileContext(nc))
        qpool = ctx.enter_context(tc.tile_pool(name="q", bufs=1))
        cpool = ctx.enter_context(tc.tile_pool(name="cand", bufs=3))
        pspool = ctx.enter_context(tc.tile_pool(name="ps", bufs=2, space="PSUM"))
        apool = ctx.enter_context(tc.tile_pool(name="accA", bufs=1))
        bpool = ctx.enter_context(tc.tile_pool(name="accB", bufs=1))
        stgpool = ctx.enter_context(tc.tile_pool(name="stg", bufs=4))

        qsb = qpool.tile([D, Q], bf16, tag="qsb")
        nc.sync.dma_start(qsb[:], qT[:])

        # A-region cell maxima: [128, NCHUNK * NA] bf16
        SA = apool.tile([P, NCHUNK * NA], bf16, tag="SA")
        # B-region running maxima: one [128, N_B, 64, 32] tile per chunk
        accB = [
            bpool.tile([P, N_B, ncell_span, CELL_A], bf16,
                       tag=f"accB_{qc}", name=f"accB_{qc}")
            for qc in range(NCHUNK)
        ]

        for t in range(repeat * NTILE):
            t = t % NTILE
            ct = cpool.tile([D, CTILE], bf16, tag="cand")
            nc.sync.dma_start(ct[:], candT[:, t * CTILE:(t + 1) * CTILE])
            for qc in range(NCHUNK):
                stg = None
                for sp in range(NSPAN):
                    ps = pspool.tile([P, ncell_span, CELL_A], f32, tag="ps")
                    for j in range(SPAN // 512):
                        col = sp * SPAN + j * 512
                        npc = 512 // CELL_A
                        nc.tensor.matmul(
                            ps[:, j * npc:(j + 1) * npc, :],
                            lhsT=qsb[:, qc * P:(qc + 1) * P],
                            rhs=ct[:, col: col + 512],
                            start=True,
                            stop=True,
                        )
                    if sp < S_A:
                        so = qc * NA + t * (S_A * ncell_span) + sp * ncell_span
                        nc.vector.tensor_reduce(
                            out=SA[:, so:so + ncell_span], in_=ps[:],
                            axis=mybir.AxisListType.X, op=mybir.AluOpType.max,
                        )
                    else:
                        sl = sp - S_A
                        if t == 0:
                            nc.scalar.activation(
                                accB[qc][:, sl], ps[:],
                                mybir.ActivationFunctionType.Copy,
                            )
                        else:
                            if stg is None:
                                stg = stgpool.tile(
                                    [P, N_B, ncell_span, CELL_A], bf16,
                                    tag="stg")
                            nc.scalar.activation(
                                stg[:, sl], ps[:],
                                mybir.ActivationFunctionType.Copy,
                            )
                            if sl == N_B - 1:
                                # one 2x-rate fold for all N_B spans at once
                                nc.vector.tensor_max(
                                    accB[qc][:], accB[qc][:], stg[:])

        for qc in range(NCHUNK):
            nc.sync.dma_start(
                out_cm[qc * P:(qc + 1) * P, :NA],
                SA[:, qc * NA:(qc + 1) * NA],
            )
            nc.sync.dma_start(
                out_cm[qc * P:(qc + 1) * P, NA:],
                accB[qc][:],
            )
    nc.compile()
    return nc


def _get_bass():
    if "nc" not in _CACHE:
        _CACHE["nc"] = _build_bass()
    return _CACHE["nc"]


def _cell_member_tables():
    """POS_A [NA, 32] and POS_B_pad [NB, 32] of core-local candidate columns
    (POS_B is padded from 16 to 32 with a huge sentinel)."""
    i = np.arange(NA)
    t = i // (S_A * (SPAN // CELL_A))
    r = i % (S_A * (SPAN // CELL_A))
    sp = r // (SPAN // CELL_A)
    c32 = r % (SPAN // CELL_A)
    baseA = t * CTILE + sp * SPAN + c32 * CELL_A
    POS_A = baseA[:, None] + np.arange(CELL_A)[None, :]

    j = np.arange(NB)
    sl = j // SPAN
    off = j % SPAN
    span = S_A + sl
    POS_B = (np.arange(NTILE)[None, :] * CTILE + (span * SPAN + off)[:, None])
    pad = np.full((NB, CELL_A - CELL_B), 1 << 40, dtype=np.int64)
    POS_B_pad = np.concatenate([POS_B.astype(np.int64), pad], axis=1)
    return POS_A.astype(np.int64), POS_B_pad


def kernel(query_embeddings, candidate_embeddings, candidate_indices, k):
    from concourse.bass_utils import run_bass_kernel_spmd

    q = np.ascontiguousarray(np.asarray(query_embeddings, dtype=np.float32))
    c = np.asarray(candidate_embeddings, dtype=np.float32).reshape(NCAND_TOTAL, D)
    ids_flat = np.asarray(candidate_indices).reshape(-1)
    k = int(k)
    assert k <= 1024

    import ml_dtypes
    bf16 = ml_dtypes.bfloat16
    qT = np.ascontiguousarray(q.T).astype(bf16)          # [128, 512]
    cT = np.ascontiguousarray(c.T.astype(bf16))          # [128, 1048576]
    in_maps = []
    for core in range(NCORES):
        in_maps.append({
            "qT": qT,
            "candT": cT[:, core * NCAND:(core + 1) * NCAND],
        })

    nc = _get_bass()
    res = run_bass_kernel_spmd(nc, in_maps, core_ids=list(range(NCORES))).results

    # ---- host: exact top-k from cell-max summaries ----
    cm = np.concatenate(
        [res[core]["out_cm"].astype(np.float32) for core in range(NCORES)],
        axis=1,
    )                                                    # [512, 8*NCELL]
    vk = np.partition(cm, -k, axis=1)[:, -k]             # kth-largest cell max
    tau = vk - MARGIN
    counts = (cm >= tau[:, None]).sum(axis=1)
    K = int(counts.max())
    sel_cells = np.argpartition(-cm, K - 1, axis=1)[:, :K]   # [512, K]

    POS_A, POS_B_pad = _cell_member_tables()
    SENT = NCAND_TOTAL                                   # dummy candidate id
    core_of = sel_cells // NCELL
    local = sel_cells - core_of * NCELL
    mA = local < NA
    pos = np.empty((Q, K, CELL_A), dtype=np.int64)
    pos[mA] = core_of[mA][:, None] * NCAND + POS_A[local[mA]]
    mB = ~mA
    pos[mB] = core_of[mB][:, None] * NCAND + POS_B_pad[local[mB] - NA]
    np.minimum(pos, SENT, out=pos)
    pos = pos.reshape(Q, K * CELL_A)

    c_ext = np.vstack([c, np.zeros((1, D), dtype=np.float32)])
    out_scores = np.empty((Q, k), dtype=np.float32)
    out_pos = np.empty((Q, k), dtype=np.int64)
    QB = 64                                              # query batch (memory cap)
    for q0 in range(0, Q, QB):
        q1 = min(q0 + QB, Q)
        sel = c_ext[pos[q0:q1]]                          # [qb, K*32, 128]
        sc = np.einsum("qnd,qd->qn", sel, q[q0:q1], optimize=True)
        for qi in range(q0, q1):
            row = sc[qi - q0]
            p = pos[qi]
            # exact order among a slightly larger head to honor tie-break
            head = np.argpartition(-row, min(k + 32, row.size - 1))[:k + 32]
            order = head[np.lexsort((p[head], -row[head]))][:k]
            out_scores[qi] = row[order]
            out_pos[qi] = p[order]

    ids_ext = np.concatenate([ids_flat, np.zeros(1, dtype=ids_flat.dtype)])
    out_ids = ids_ext[out_pos].astype(ids_flat.dtype)
    return out_scores, out_ids
